# revision 1
# baseline (speedup 1.0000x reference)
"""Trainium2 Bass kernel for nn_AverageCrossStitch (bidirectional cross-attention).

reference:
    S = input1 @ input2^T / sqrt(D)          # [b, i, j]
    out1 = 0.5*input1 + 0.5*softmax_j(S) @ input2
    out2 = 0.5*input2 + 0.5*softmax_i(S)^T @ input1

Sharding: data-parallel over batch, one batch element per NeuronCore (B=8, 8 cores).

Per-core algorithm (all matmuls bf16, fp32 residual; no-max softmax since
scores ~ N(0,1), |score| < ~6 so exp cannot overflow):

  Phase 1 (produces out1):
    for each i-block (512 cols):
      for each j-tile (128 rows):  ET[j,i] = exp(X2 X1^T / 32) via
        psum = sum_d lhsT(X2T d-tile).T @ rhs(X1T i-block)  -> [j=128, i=512]
        ACT exp -> bf16 ET tile; accum_out gives sum_i exp (= phase-2 denominators)
      PV: for each i-subtile (128), d-block (512):
        psum_o = sum_jt lhsT(ET[:, i128]).T @ rhs(X2 natural) -> [i, d]
        denominator: same lhsT vs a constant 2.0 column -> psum_r[i,1] = 2*rowsum
        out = (psum_o * reciprocal(psum_r)) + 0.5*X1   (one fused DVE op)
  Phase 2: mirror image with roles of X1/X2 (and i/j) swapped; its softmax
  denominators were accumulated during phase 1's exps (accum_out), since the
  shared score matrix is just transposed between the passes.
"""

import numpy as np
import ml_dtypes

import concourse.bass as bass
import concourse.bacc as bacc
import concourse.mybir as mybir
import concourse.tile as tile
from concourse.bass_utils import run_bass_kernel_spmd

P = 128  # SBUF partitions

F32 = mybir.dt.float32
BF16 = mybir.dt.bfloat16
AF = mybir.ActivationFunctionType
ALU = mybir.AluOpType


def build_body(nc, tc, S, D, NB=512):
    """Emit the per-core kernel body. S: sequence length, D: model dim,
    NB: free-dim block (<= 512 to fit one PSUM bank in fp32)."""
    assert S % NB == 0 and D % NB == 0 and S % P == 0 and D % P == 0
    nT = S // P    # seq tiles of 128
    nDT = D // P   # contraction tiles of 128
    nIB = S // NB  # seq blocks of NB
    nDB = D // NB  # d blocks of NB
    nSUB = NB // P # 128-subtiles per seq block
    scale = 1.0 / float(np.sqrt(D))

    x1t = nc.dram_tensor("x1t", [D, S], BF16, kind="ExternalInput")
    x2t = nc.dram_tensor("x2t", [D, S], BF16, kind="ExternalInput")
    x1n = nc.dram_tensor("x1n", [S, D], BF16, kind="ExternalInput")
    x2n = nc.dram_tensor("x2n", [S, D], BF16, kind="ExternalInput")
    x1h = nc.dram_tensor("x1h", [S, D], F32, kind="ExternalInput")  # 0.5*X1
    x2h = nc.dram_tensor("x2h", [S, D], F32, kind="ExternalInput")  # 0.5*X2
    o1 = nc.dram_tensor("o1", [S, D], F32, kind="ExternalOutput")
    o2 = nc.dram_tensor("o2", [S, D], F32, kind="ExternalOutput")

    with (
        tc.tile_pool(name="p_x1t", bufs=nDT) as p_x1t,
        tc.tile_pool(name="p_x2t", bufs=nDT) as p_x2t,
        tc.tile_pool(name="p_x1n", bufs=nT) as p_x1n,
        tc.tile_pool(name="p_x2n", bufs=nT) as p_x2n,
        tc.tile_pool(name="p_e1", bufs=min(nT + 4, 2 * nT)) as p_e1,
        tc.tile_pool(name="p_e2", bufs=min(nT + 4, 2 * nT)) as p_e2,
        tc.tile_pool(name="p_h", bufs=3) as p_h,
        tc.tile_pool(name="p_out", bufs=4) as p_out,
        tc.tile_pool(name="p_small", bufs=8) as p_small,
        tc.tile_pool(name="p_const", bufs=2) as p_const,
        tc.tile_pool(name="ps_sc", bufs=3, space=bass.MemorySpace.PSUM) as ps_sc,
        tc.tile_pool(name="ps_pv", bufs=3, space=bass.MemorySpace.PSUM) as ps_pv,
        tc.tile_pool(name="ps_dn", bufs=2, space=bass.MemorySpace.PSUM) as ps_dn,
    ):
        # ---- constants and resident inputs ----
        twos = p_const.tile([P, 1], BF16, tag="twos")
        nc.vector.memset(twos[:], 2.0)
        # phase-2 denominator partials: [P, jt, ib] accumulated by phase-1 exps
        d2p = p_const.tile([P, nT, nIB], F32, tag="d2p")

        sb_x1t = []
        sb_x2t = []
        for dt in range(nDT):
            t1 = p_x1t.tile([P, S], BF16, tag="x1t")
            nc.sync.dma_start(t1[:], x1t[dt * P:(dt + 1) * P, :])
            sb_x1t.append(t1)
            t2 = p_x2t.tile([P, S], BF16, tag="x2t")
            nc.sync.dma_start(t2[:], x2t[dt * P:(dt + 1) * P, :])
            sb_x2t.append(t2)
        sb_x2n = []
        for jt in range(nT):
            t = p_x2n.tile([P, D], BF16, tag="x2n")
            nc.sync.dma_start(t[:], x2n[jt * P:(jt + 1) * P, :])
            sb_x2n.append(t)
        sb_x1n = []
        for it in range(nT):
            t = p_x1n.tile([P, D], BF16, tag="x1n")
            nc.sync.dma_start(t[:], x1n[it * P:(it + 1) * P, :])
            sb_x1n.append(t)

        # ---------------- phase 1: out1 ----------------
        for ib in range(nIB):
            isl = slice(ib * NB, (ib + 1) * NB)
            e1_tiles = []
            for jt in range(nT):
                ps = ps_sc.tile([P, NB], F32, tag="sc")
                for dt in range(nDT):
                    nc.tensor.matmul(
                        ps[:],
                        sb_x2t[dt][:, jt * P:(jt + 1) * P],
                        sb_x1t[dt][:, isl],
                        start=(dt == 0),
                        stop=(dt == nDT - 1),
                    )
                et = p_e1.tile([P, NB], BF16, tag="e1")
                nc.scalar.activation(
                    et[:], ps[:], AF.Exp, scale=scale,
                    accum_out=d2p[:, jt, ib:ib + 1],
                )
                e1_tiles.append(et)
            for sub in range(nSUB):
                it = ib * nSUB + sub
                ssl = slice(sub * P, (sub + 1) * P)
                ht = p_h.tile([P, D], F32, tag="h")
                nc.sync.dma_start(ht[:], x1h[it * P:(it + 1) * P, :])
                ps_r = ps_dn.tile([P, 1], F32, tag="dn")
                for jt in range(nT):
                    nc.tensor.matmul(
                        ps_r[:], e1_tiles[jt][:, ssl], twos[:],
                        start=(jt == 0), stop=(jt == nT - 1),
                    )
                r1 = p_small.tile([P, 1], F32, tag="r")
                nc.vector.reciprocal(r1[:], ps_r[:])  # = 0.5 / rowsum
                for db in range(nDB):
                    dsl = slice(db * NB, (db + 1) * NB)
                    ps_o = ps_pv.tile([P, NB], F32, tag="pv")
                    for jt in range(nT):
                        nc.tensor.matmul(
                            ps_o[:], e1_tiles[jt][:, ssl], sb_x2n[jt][:, dsl],
                            start=(jt == 0), stop=(jt == nT - 1),
                        )
                    ob = p_out.tile([P, NB], F32, tag="ob")
                    nc.vector.scalar_tensor_tensor(
                        ob[:], ps_o[:], r1[:], ht[:, dsl],
                        op0=ALU.mult, op1=ALU.add,
                    )
                    nc.sync.dma_start(o1[it * P:(it + 1) * P, dsl], ob[:])

        # ---------------- phase 2: out2 (mirror) ----------------
        for jb in range(nIB):
            jsl = slice(jb * NB, (jb + 1) * NB)
            e2_tiles = []
            for it in range(nT):
                ps = ps_sc.tile([P, NB], F32, tag="sc")
                for dt in range(nDT):
                    nc.tensor.matmul(
                        ps[:],
                        sb_x1t[dt][:, it * P:(it + 1) * P],
                        sb_x2t[dt][:, jsl],
                        start=(dt == 0),
                        stop=(dt == nDT - 1),
                    )
                e2 = p_e2.tile([P, NB], BF16, tag="e2")
                nc.scalar.activation(e2[:], ps[:], AF.Exp, scale=scale)
                e2_tiles.append(e2)
            for sub in range(nSUB):
                jt = jb * nSUB + sub
                ssl = slice(sub * P, (sub + 1) * P)
                ht = p_h.tile([P, D], F32, tag="h")
                nc.sync.dma_start(ht[:], x2h[jt * P:(jt + 1) * P, :])
                # denom2 = sum over ib of phase-1 accum partials; r2 = 0.5/denom2
                dn2 = p_small.tile([P, 1], F32, tag="dn2")
                nc.vector.tensor_reduce(
                    dn2[:], d2p[:, jt, :], axis=mybir.AxisListType.X, op=ALU.add,
                )
                dn2x2 = p_small.tile([P, 1], F32, tag="dn2x2")
                nc.vector.tensor_scalar_mul(dn2x2[:], dn2[:], 2.0)
                r2 = p_small.tile([P, 1], F32, tag="r2")
                nc.vector.reciprocal(r2[:], dn2x2[:])
                for db in range(nDB):
                    dsl = slice(db * NB, (db + 1) * NB)
                    ps_o = ps_pv.tile([P, NB], F32, tag="pv")
                    for it in range(nT):
                        nc.tensor.matmul(
                            ps_o[:], e2_tiles[it][:, ssl], sb_x1n[it][:, dsl],
                            start=(it == 0), stop=(it == nT - 1),
                        )
                    ob = p_out.tile([P, NB], F32, tag="ob")
                    nc.vector.scalar_tensor_tensor(
                        ob[:], ps_o[:], r2[:], ht[:, dsl],
                        op0=ALU.mult, op1=ALU.add,
                    )
                    nc.sync.dma_start(o2[jt * P:(jt + 1) * P, dsl], ob[:])


def build_nc(S=2048, D=1024, NB=512, n_cores=8):
    nc = bacc.Bacc(
        "TRN2",
        target_bir_lowering=False,
        debug=False,
        enable_asserts=False,
        num_devices=n_cores,
    )
    with tile.TileContext(nc) as tc:
        build_body(nc, tc, S, D, NB)
    nc.compile()
    return nc


def make_in_map(x1, x2):
    """Host-side prep of one batch element's per-core inputs."""
    x1 = np.ascontiguousarray(x1, dtype=np.float32)
    x2 = np.ascontiguousarray(x2, dtype=np.float32)
    return {
        "x1t": np.ascontiguousarray(x1.T).astype(ml_dtypes.bfloat16),
        "x2t": np.ascontiguousarray(x2.T).astype(ml_dtypes.bfloat16),
        "x1n": x1.astype(ml_dtypes.bfloat16),
        "x2n": x2.astype(ml_dtypes.bfloat16),
        "x1h": (0.5 * x1).astype(np.float32),
        "x2h": (0.5 * x2).astype(np.float32),
    }


_NC_CACHE = {}


def _get_nc(S, D, n_cores):
    key = (S, D, n_cores)
    if key not in _NC_CACHE:
        _NC_CACHE[key] = build_nc(S=S, D=D, n_cores=n_cores)
    return _NC_CACHE[key]


def kernel(layer_key=None, input1=None, input2=None, _trace=False, **_ignored):
    X1 = np.asarray(input1, dtype=np.float32)
    X2 = np.asarray(input2, dtype=np.float32)
    B, S, D = X1.shape
    n_cores = 8
    assert B == n_cores, f"expected batch {n_cores}, got {B}"

    nc = _get_nc(S, D, n_cores)
    in_maps = [make_in_map(X1[b], X2[b]) for b in range(B)]
    res = run_bass_kernel_spmd(
        nc, in_maps, core_ids=list(range(n_cores)),
        trace=_trace, trace_cores=[0] if _trace else None,
    )
    out1 = np.stack([res.results[b]["o1"] for b in range(B)])
    out2 = np.stack([res.results[b]["o2"] for b in range(B)])
    if _trace:
        kernel.last_results = res
    return (out1, out2)


# revision 6
# speedup vs baseline: 12.0749x; 12.0749x over previous
"""Trainium2 Bass kernel for nn_AverageCrossStitch (bidirectional cross-attention).

reference:
    S = input1 @ input2^T / sqrt(D)          # [b, i, j]
    out1 = 0.5*input1 + 0.5*softmax_j(S) @ input2
    out2 = 0.5*input2 + 0.5*softmax_i(S)^T @ input1

Sharding: data-parallel over batch, one batch element per NeuronCore (B=8, 8 cores).

Per-core algorithm (all matmuls bf16, fp32 residual; no-max softmax since
scores ~ N(0,1), |score| < ~6 so exp cannot overflow):

  Phase 1 (produces out1):
    for each i-block (512 cols):
      for each j-tile (128 rows):  ET[j,i] = exp(X2 X1^T / 32) via
        psum = sum_d lhsT(X2T d-tile).T @ rhs(X1T i-block)  -> [j=128, i=512]
        ACT exp -> bf16 ET tile; accum_out gives sum_i exp (= phase-2 denominators)
      PV: for each i-subtile (128), d-block (512):
        psum_o = sum_jt lhsT(ET[:, i128]).T @ rhs(X2 natural) -> [i, d]
        denominator: same lhsT vs a constant 2.0 column -> psum_r[i,1] = 2*rowsum
        out = (psum_o * reciprocal(psum_r)) + 0.5*X1   (one fused DVE op)
  Phase 2: mirror image with roles of X1/X2 (and i/j) swapped; its softmax
  denominators were accumulated during phase 1's exps (accum_out), since the
  shared score matrix is just transposed between the passes.
"""

import numpy as np
import ml_dtypes

import concourse.bass as bass
import concourse.bacc as bacc
import concourse.mybir as mybir
import concourse.tile as tile
from concourse.bass_utils import run_bass_kernel_spmd

P = 128  # SBUF partitions

F32 = mybir.dt.float32
BF16 = mybir.dt.bfloat16
AF = mybir.ActivationFunctionType
ALU = mybir.AluOpType


def declare_io(nc, S, D):
    return {
        "x1t": nc.dram_tensor("x1t", [D, S], BF16, kind="ExternalInput"),
        "x2t": nc.dram_tensor("x2t", [D, S], BF16, kind="ExternalInput"),
        "x1n": nc.dram_tensor("x1n", [S, D], BF16, kind="ExternalInput"),
        "x2n": nc.dram_tensor("x2n", [S, D], BF16, kind="ExternalInput"),
        "x1h": nc.dram_tensor("x1h", [S, D], F32, kind="ExternalInput"),  # 0.5*X1
        "x2h": nc.dram_tensor("x2h", [S, D], F32, kind="ExternalInput"),  # 0.5*X2
        "o1": nc.dram_tensor("o1", [S, D], F32, kind="ExternalOutput"),
        "o2": nc.dram_tensor("o2", [S, D], F32, kind="ExternalOutput"),
    }


def build_body(nc, tc, S, D, NB=512, io=None):
    """Emit the per-core kernel body. S: sequence length, D: model dim,
    NB: free-dim block (<= 512 to fit one PSUM bank in fp32)."""
    assert S % NB == 0 and D % NB == 0 and S % P == 0 and D % P == 0
    nT = S // P    # seq tiles of 128
    nDT = D // P   # contraction tiles of 128
    nIB = S // NB  # seq blocks of NB
    nDB = D // NB  # d blocks of NB
    nSUB = NB // P # 128-subtiles per seq block
    scale = 1.0 / float(np.sqrt(D))

    if io is None:
        io = declare_io(nc, S, D)
    x1t, x2t, x1n, x2n, x1h, x2h, o1, o2 = (
        io["x1t"], io["x2t"], io["x1n"], io["x2n"],
        io["x1h"], io["x2h"], io["o1"], io["o2"],
    )

    with (
        tc.tile_pool(name="p_x1t", bufs=nDT) as p_x1t,
        tc.tile_pool(name="p_x2t", bufs=nDT) as p_x2t,
        tc.tile_pool(name="p_x1n", bufs=nT) as p_x1n,
        tc.tile_pool(name="p_x2n", bufs=nT) as p_x2n,
        tc.tile_pool(name="p_e1", bufs=min(nT + 4, 2 * nT)) as p_e1,
        tc.tile_pool(name="p_e2", bufs=min(nT + 4, 2 * nT)) as p_e2,
        tc.tile_pool(name="p_h", bufs=3) as p_h,
        tc.tile_pool(name="p_out", bufs=4) as p_out,
        tc.tile_pool(name="p_small", bufs=8) as p_small,
        tc.tile_pool(name="p_const", bufs=2) as p_const,
        tc.tile_pool(name="ps_sc", bufs=3, space=bass.MemorySpace.PSUM) as ps_sc,
        tc.tile_pool(name="ps_pv", bufs=3, space=bass.MemorySpace.PSUM) as ps_pv,
        tc.tile_pool(name="ps_dn", bufs=2, space=bass.MemorySpace.PSUM) as ps_dn,
    ):
        # ---- constants and resident inputs ----
        twos = p_const.tile([P, 1], BF16, tag="twos")
        nc.vector.memset(twos[:], 2.0)
        # phase-2 denominator partials: [P, jt, ib] accumulated by phase-1 exps
        d2p = p_const.tile([P, nT, nIB], F32, tag="d2p")

        sb_x1t = []
        sb_x2t = []
        for dt in range(nDT):
            t1 = p_x1t.tile([P, S], BF16, tag="x1t")
            nc.sync.dma_start(t1[:], x1t[dt * P:(dt + 1) * P, :])
            sb_x1t.append(t1)
            t2 = p_x2t.tile([P, S], BF16, tag="x2t")
            nc.sync.dma_start(t2[:], x2t[dt * P:(dt + 1) * P, :])
            sb_x2t.append(t2)
        sb_x2n = []
        for jt in range(nT):
            t = p_x2n.tile([P, D], BF16, tag="x2n")
            nc.sync.dma_start(t[:], x2n[jt * P:(jt + 1) * P, :])
            sb_x2n.append(t)
        sb_x1n = []
        for it in range(nT):
            t = p_x1n.tile([P, D], BF16, tag="x1n")
            nc.sync.dma_start(t[:], x1n[it * P:(it + 1) * P, :])
            sb_x1n.append(t)

        # ---------------- phase 1: out1 ----------------
        for ib in range(nIB):
            isl = slice(ib * NB, (ib + 1) * NB)
            e1_tiles = []
            for jt in range(nT):
                ps = ps_sc.tile([P, NB], F32, tag="sc")
                for dt in range(nDT):
                    nc.tensor.matmul(
                        ps[:],
                        sb_x2t[dt][:, jt * P:(jt + 1) * P],
                        sb_x1t[dt][:, isl],
                        start=(dt == 0),
                        stop=(dt == nDT - 1),
                    )
                et = p_e1.tile([P, NB], BF16, tag="e1")
                nc.scalar.activation(
                    et[:], ps[:], AF.Exp, scale=scale,
                    accum_out=d2p[:, jt, ib:ib + 1],
                )
                e1_tiles.append(et)
            for sub in range(nSUB):
                it = ib * nSUB + sub
                ssl = slice(sub * P, (sub + 1) * P)
                ht = p_h.tile([P, D], F32, tag="h")
                nc.sync.dma_start(ht[:], x1h[it * P:(it + 1) * P, :])
                ps_r = ps_dn.tile([P, 1], F32, tag="dn")
                for jt in range(nT):
                    nc.tensor.matmul(
                        ps_r[:], e1_tiles[jt][:, ssl], twos[:],
                        start=(jt == 0), stop=(jt == nT - 1),
                    )
                r1 = p_small.tile([P, 1], F32, tag="r")
                nc.vector.reciprocal(r1[:], ps_r[:])  # = 0.5 / rowsum
                for db in range(nDB):
                    dsl = slice(db * NB, (db + 1) * NB)
                    ps_o = ps_pv.tile([P, NB], F32, tag="pv")
                    for jt in range(nT):
                        nc.tensor.matmul(
                            ps_o[:], e1_tiles[jt][:, ssl], sb_x2n[jt][:, dsl],
                            start=(jt == 0), stop=(jt == nT - 1),
                        )
                    ob = p_out.tile([P, NB], F32, tag="ob")
                    nc.vector.scalar_tensor_tensor(
                        ob[:], ps_o[:], r1[:], ht[:, dsl],
                        op0=ALU.mult, op1=ALU.add,
                    )
                    nc.sync.dma_start(o1[it * P:(it + 1) * P, dsl], ob[:])

        # ---------------- phase 2: out2 (mirror) ----------------
        for jb in range(nIB):
            jsl = slice(jb * NB, (jb + 1) * NB)
            e2_tiles = []
            for it in range(nT):
                ps = ps_sc.tile([P, NB], F32, tag="sc")
                for dt in range(nDT):
                    nc.tensor.matmul(
                        ps[:],
                        sb_x1t[dt][:, it * P:(it + 1) * P],
                        sb_x2t[dt][:, jsl],
                        start=(dt == 0),
                        stop=(dt == nDT - 1),
                    )
                e2 = p_e2.tile([P, NB], BF16, tag="e2")
                nc.scalar.activation(e2[:], ps[:], AF.Exp, scale=scale)
                e2_tiles.append(e2)
            for sub in range(nSUB):
                jt = jb * nSUB + sub
                ssl = slice(sub * P, (sub + 1) * P)
                ht = p_h.tile([P, D], F32, tag="h")
                nc.sync.dma_start(ht[:], x2h[jt * P:(jt + 1) * P, :])
                # denom2 = sum over ib of phase-1 accum partials; r2 = 0.5/denom2
                dn2 = p_small.tile([P, 1], F32, tag="dn2")
                nc.vector.tensor_reduce(
                    dn2[:], d2p[:, jt, :], axis=mybir.AxisListType.X, op=ALU.add,
                )
                dn2x2 = p_small.tile([P, 1], F32, tag="dn2x2")
                nc.vector.tensor_scalar_mul(dn2x2[:], dn2[:], 2.0)
                r2 = p_small.tile([P, 1], F32, tag="r2")
                nc.vector.reciprocal(r2[:], dn2x2[:])
                for db in range(nDB):
                    dsl = slice(db * NB, (db + 1) * NB)
                    ps_o = ps_pv.tile([P, NB], F32, tag="pv")
                    for it in range(nT):
                        nc.tensor.matmul(
                            ps_o[:], e2_tiles[it][:, ssl], sb_x1n[it][:, dsl],
                            start=(it == 0), stop=(it == nT - 1),
                        )
                    ob = p_out.tile([P, NB], F32, tag="ob")
                    nc.vector.scalar_tensor_tensor(
                        ob[:], ps_o[:], r2[:], ht[:, dsl],
                        op0=ALU.mult, op1=ALU.add,
                    )
                    nc.sync.dma_start(o2[jt * P:(jt + 1) * P, dsl], ob[:])


def build_body_v2(nc, tc, S, D, NB=512, io=None):
    """v2: compute scores/exp once (as E^T), produce phase-2's E-natural tiles
    via DMA xbar transpose instead of recomputing the score matmuls.

    PE work drops from 4 to 3 big matmul sets (scores once + two PV passes);
    the transpose rides on the DMA engines (~16MB through the xbar).
    """
    assert S % NB == 0 and D % NB == 0 and S % P == 0 and D % P == 0
    nT = S // P
    nDT = D // P
    nIB = S // NB
    nDB = D // NB
    nSUB = NB // P
    scale = 1.0 / float(np.sqrt(D))

    if io is None:
        io = declare_io(nc, S, D)
    x1t, x2t, x1n, x2n, x1h, x2h, o1, o2 = (
        io["x1t"], io["x2t"], io["x1n"], io["x2n"],
        io["x1h"], io["x2h"], io["o1"], io["o2"],
    )

    with (
        tc.tile_pool(name="p_et", bufs=nT) as p_et,
        tc.tile_pool(name="p_h", bufs=3) as p_h,
        tc.tile_pool(name="p_out", bufs=4) as p_out,
        tc.tile_pool(name="p_small", bufs=8) as p_small,
        tc.tile_pool(name="p_const", bufs=2) as p_const,
        tc.tile_pool(name="ps_sc", bufs=3, space=bass.MemorySpace.PSUM) as ps_sc,
        tc.tile_pool(name="ps_pv", bufs=3, space=bass.MemorySpace.PSUM) as ps_pv,
        tc.tile_pool(name="ps_dn", bufs=2, space=bass.MemorySpace.PSUM) as ps_dn,
    ):
        twos = p_const.tile([P, 1], BF16, tag="twos")
        nc.vector.memset(twos[:], 2.0)
        d2p = p_const.tile([P, nT, nIB], F32, tag="d2p")

        # E^T store: per j-tile, all i columns (written slice-wise by the exps)
        sb_et = [p_et.tile([P, S], BF16, tag="et") for _ in range(nT)]

        # ---------------- phase 1 (+ E^T materialization) ----------------
        with (
            tc.tile_pool(name="p_x1t", bufs=nDT) as p_x1t,
            tc.tile_pool(name="p_x2t", bufs=nDT) as p_x2t,
            tc.tile_pool(name="p_x2n", bufs=nT) as p_x2n,
        ):
            sb_x1t, sb_x2t = [], []
            for dt in range(nDT):
                t1 = p_x1t.tile([P, S], BF16, tag="x1t")
                nc.sync.dma_start(t1[:], x1t[dt * P:(dt + 1) * P, :])
                sb_x1t.append(t1)
                t2 = p_x2t.tile([P, S], BF16, tag="x2t")
                nc.sync.dma_start(t2[:], x2t[dt * P:(dt + 1) * P, :])
                sb_x2t.append(t2)
            sb_x2n = []
            for jt in range(nT):
                t = p_x2n.tile([P, D], BF16, tag="x2n")
                nc.sync.dma_start(t[:], x2n[jt * P:(jt + 1) * P, :])
                sb_x2n.append(t)

            for ib in range(nIB):
                isl = slice(ib * NB, (ib + 1) * NB)
                for jt in range(nT):
                    ps = ps_sc.tile([P, NB], F32, tag="sc")
                    for dt in range(nDT):
                        nc.tensor.matmul(
                            ps[:],
                            sb_x2t[dt][:, jt * P:(jt + 1) * P],
                            sb_x1t[dt][:, isl],
                            start=(dt == 0),
                            stop=(dt == nDT - 1),
                        )
                    nc.scalar.activation(
                        sb_et[jt][:, isl], ps[:], AF.Exp, scale=scale,
                        accum_out=d2p[:, jt, ib:ib + 1],
                    )
                for sub in range(nSUB):
                    it = ib * nSUB + sub
                    csl = slice(it * P, (it + 1) * P)
                    ht = p_h.tile([P, D], F32, tag="h")
                    nc.sync.dma_start(ht[:], x1h[it * P:(it + 1) * P, :])
                    ps_r = ps_dn.tile([P, 1], F32, tag="dn")
                    for jt in range(nT):
                        nc.tensor.matmul(
                            ps_r[:], sb_et[jt][:, csl], twos[:],
                            start=(jt == 0), stop=(jt == nT - 1),
                        )
                    r1 = p_small.tile([P, 1], F32, tag="r")
                    nc.vector.reciprocal(r1[:], ps_r[:])  # = 0.5 / rowsum
                    for db in range(nDB):
                        dsl = slice(db * NB, (db + 1) * NB)
                        ps_o = ps_pv.tile([P, NB], F32, tag="pv")
                        for jt in range(nT):
                            nc.tensor.matmul(
                                ps_o[:], sb_et[jt][:, csl], sb_x2n[jt][:, dsl],
                                start=(jt == 0), stop=(jt == nT - 1),
                            )
                        ob = p_out.tile([P, NB], F32, tag="ob")
                        nc.vector.scalar_tensor_tensor(
                            ob[:], ps_o[:], r1[:], ht[:, dsl],
                            op0=ALU.mult, op1=ALU.add,
                        )
                        nc.sync.dma_start(o1[it * P:(it + 1) * P, dsl], ob[:])

        # ---------------- phase 2: transpose E^T, PV with X1 ----------------
        with (
            tc.tile_pool(name="p_x1n", bufs=nT) as p_x1n,
            tc.tile_pool(name="p_en", bufs=3) as p_en,
        ):
            sb_x1n = []
            for it in range(nT):
                t = p_x1n.tile([P, D], BF16, tag="x1n")
                nc.sync.dma_start(t[:], x1n[it * P:(it + 1) * P, :])
                sb_x1n.append(t)

            for jt in range(nT):
                # e_nat[p_i, kt, f_j] = E[kt*128+p_i, jt*128+f_j]
                en = p_en.tile([P, nT, P], BF16, tag="en")
                nc.sync.dma_start_transpose(en[:], sb_et[jt][:])

                dn2 = p_small.tile([P, 1], F32, tag="dn2")
                nc.vector.tensor_reduce(
                    dn2[:], d2p[:, jt, :], axis=mybir.AxisListType.X, op=ALU.add,
                )
                dn2x2 = p_small.tile([P, 1], F32, tag="dn2x2")
                nc.vector.tensor_scalar_mul(dn2x2[:], dn2[:], 2.0)
                r2 = p_small.tile([P, 1], F32, tag="r2")
                nc.vector.reciprocal(r2[:], dn2x2[:])  # = 0.5 / colsum

                ht = p_h.tile([P, D], F32, tag="h")
                nc.sync.dma_start(ht[:], x2h[jt * P:(jt + 1) * P, :])
                for db in range(nDB):
                    dsl = slice(db * NB, (db + 1) * NB)
                    ps_o = ps_pv.tile([P, NB], F32, tag="pv")
                    for it in range(nT):
                        nc.tensor.matmul(
                            ps_o[:], en[:, it, :], sb_x1n[it][:, dsl],
                            start=(it == 0), stop=(it == nT - 1),
                        )
                    ob = p_out.tile([P, NB], F32, tag="ob")
                    nc.vector.scalar_tensor_tensor(
                        ob[:], ps_o[:], r2[:], ht[:, dsl],
                        op0=ALU.mult, op1=ALU.add,
                    )
                    nc.sync.dma_start(o2[jt * P:(jt + 1) * P, dsl], ob[:])


BODY_VERSION = 2


def build_nc(S=2048, D=1024, NB=512, n_cores=8, repeats=1, version=None):
    nc = bacc.Bacc(
        "TRN2",
        target_bir_lowering=False,
        debug=False,
        enable_asserts=False,
        num_devices=n_cores,
    )
    if version is None:
        version = BODY_VERSION
    body = {1: build_body, 2: build_body_v2}[version]
    with tile.TileContext(nc) as tc:
        io = declare_io(nc, S, D)
        # benchmark mode (repeats > 1): emit the body R times back-to-back so
        # per-execution device time can be measured as a slope over R
        for _ in range(repeats):
            body(nc, tc, S, D, NB, io=io)
    nc.compile()
    return nc


def make_in_map(x1, x2):
    """Host-side prep of one batch element's per-core inputs."""
    x1 = np.ascontiguousarray(x1, dtype=np.float32)
    x2 = np.ascontiguousarray(x2, dtype=np.float32)
    return {
        "x1t": np.ascontiguousarray(x1.T).astype(ml_dtypes.bfloat16),
        "x2t": np.ascontiguousarray(x2.T).astype(ml_dtypes.bfloat16),
        "x1n": x1.astype(ml_dtypes.bfloat16),
        "x2n": x2.astype(ml_dtypes.bfloat16),
        "x1h": (0.5 * x1).astype(np.float32),
        "x2h": (0.5 * x2).astype(np.float32),
    }


_NC_CACHE = {}


def _get_nc(S, D, n_cores):
    key = (S, D, n_cores)
    if key not in _NC_CACHE:
        _NC_CACHE[key] = build_nc(S=S, D=D, n_cores=n_cores)
    return _NC_CACHE[key]


def kernel(layer_key=None, input1=None, input2=None, _trace=False, **_ignored):
    X1 = np.asarray(input1, dtype=np.float32)
    X2 = np.asarray(input2, dtype=np.float32)
    B, S, D = X1.shape
    n_cores = 8
    assert B == n_cores, f"expected batch {n_cores}, got {B}"

    nc = _get_nc(S, D, n_cores)
    in_maps = [make_in_map(X1[b], X2[b]) for b in range(B)]
    res = run_bass_kernel_spmd(
        nc, in_maps, core_ids=list(range(n_cores)),
        trace=_trace, trace_cores=[0] if _trace else None,
    )
    out1 = np.stack([res.results[b]["o1"] for b in range(B)])
    out2 = np.stack([res.results[b]["o2"] for b in range(B)])
    if _trace:
        kernel.last_results = res
    return (out1, out2)


# revision 9
# speedup vs baseline: 17.3220x; 1.4346x over previous
"""Trainium2 Bass kernel for nn_AverageCrossStitch (bidirectional cross-attention).

reference:
    S = input1 @ input2^T / sqrt(D)          # [b, i, j]
    out1 = 0.5*input1 + 0.5*softmax_j(S) @ input2
    out2 = 0.5*input2 + 0.5*softmax_i(S)^T @ input1

Sharding: data-parallel over batch, one batch element per NeuronCore (B=8, 8 cores).

Per-core algorithm (all matmuls bf16, fp32 residual; no-max softmax since
scores ~ N(0,1), |score| < ~6 so exp cannot overflow):

  Phase 1 (produces out1):
    for each i-block (512 cols):
      for each j-tile (128 rows):  ET[j,i] = exp(X2 X1^T / 32) via
        psum = sum_d lhsT(X2T d-tile).T @ rhs(X1T i-block)  -> [j=128, i=512]
        ACT exp -> bf16 ET tile; accum_out gives sum_i exp (= phase-2 denominators)
      PV: for each i-subtile (128), d-block (512):
        psum_o = sum_jt lhsT(ET[:, i128]).T @ rhs(X2 natural) -> [i, d]
        denominator: same lhsT vs a constant 2.0 column -> psum_r[i,1] = 2*rowsum
        out = (psum_o * reciprocal(psum_r)) + 0.5*X1   (one fused DVE op)
  Phase 2: mirror image with roles of X1/X2 (and i/j) swapped; its softmax
  denominators were accumulated during phase 1's exps (accum_out), since the
  shared score matrix is just transposed between the passes.
"""

import numpy as np
import ml_dtypes

import concourse.bass as bass
import concourse.bacc as bacc
import concourse.mybir as mybir
import concourse.tile as tile
from concourse.bass_utils import run_bass_kernel_spmd

P = 128  # SBUF partitions

F32 = mybir.dt.float32
BF16 = mybir.dt.bfloat16
AF = mybir.ActivationFunctionType
ALU = mybir.AluOpType


def declare_io(nc, S, D):
    return {
        "x1t": nc.dram_tensor("x1t", [D, S], BF16, kind="ExternalInput"),
        "x2t": nc.dram_tensor("x2t", [D, S], BF16, kind="ExternalInput"),
        "x1n": nc.dram_tensor("x1n", [S, D], BF16, kind="ExternalInput"),
        "x2n": nc.dram_tensor("x2n", [S, D], BF16, kind="ExternalInput"),
        "x1h": nc.dram_tensor("x1h", [S, D], F32, kind="ExternalInput"),  # 0.5*X1
        "x2h": nc.dram_tensor("x2h", [S, D], F32, kind="ExternalInput"),  # 0.5*X2
        "o1": nc.dram_tensor("o1", [S, D], F32, kind="ExternalOutput"),
        "o2": nc.dram_tensor("o2", [S, D], F32, kind="ExternalOutput"),
    }


def build_body(nc, tc, S, D, NB=512, io=None):
    """Emit the per-core kernel body. S: sequence length, D: model dim,
    NB: free-dim block (<= 512 to fit one PSUM bank in fp32)."""
    assert S % NB == 0 and D % NB == 0 and S % P == 0 and D % P == 0
    nT = S // P    # seq tiles of 128
    nDT = D // P   # contraction tiles of 128
    nIB = S // NB  # seq blocks of NB
    nDB = D // NB  # d blocks of NB
    nSUB = NB // P # 128-subtiles per seq block
    scale = 1.0 / float(np.sqrt(D))

    if io is None:
        io = declare_io(nc, S, D)
    x1t, x2t, x1n, x2n, x1h, x2h, o1, o2 = (
        io["x1t"], io["x2t"], io["x1n"], io["x2n"],
        io["x1h"], io["x2h"], io["o1"], io["o2"],
    )

    with (
        tc.tile_pool(name="p_x1t", bufs=nDT) as p_x1t,
        tc.tile_pool(name="p_x2t", bufs=nDT) as p_x2t,
        tc.tile_pool(name="p_x1n", bufs=nT) as p_x1n,
        tc.tile_pool(name="p_x2n", bufs=nT) as p_x2n,
        tc.tile_pool(name="p_e1", bufs=min(nT + 4, 2 * nT)) as p_e1,
        tc.tile_pool(name="p_e2", bufs=min(nT + 4, 2 * nT)) as p_e2,
        tc.tile_pool(name="p_h", bufs=3) as p_h,
        tc.tile_pool(name="p_out", bufs=4) as p_out,
        tc.tile_pool(name="p_small", bufs=8) as p_small,
        tc.tile_pool(name="p_const", bufs=2) as p_const,
        tc.tile_pool(name="ps_sc", bufs=3, space=bass.MemorySpace.PSUM) as ps_sc,
        tc.tile_pool(name="ps_pv", bufs=3, space=bass.MemorySpace.PSUM) as ps_pv,
        tc.tile_pool(name="ps_dn", bufs=2, space=bass.MemorySpace.PSUM) as ps_dn,
    ):
        # ---- constants and resident inputs ----
        twos = p_const.tile([P, 1], BF16, tag="twos")
        nc.vector.memset(twos[:], 2.0)
        # phase-2 denominator partials: [P, jt, ib] accumulated by phase-1 exps
        d2p = p_const.tile([P, nT, nIB], F32, tag="d2p")

        sb_x1t = []
        sb_x2t = []
        for dt in range(nDT):
            t1 = p_x1t.tile([P, S], BF16, tag="x1t")
            nc.sync.dma_start(t1[:], x1t[dt * P:(dt + 1) * P, :])
            sb_x1t.append(t1)
            t2 = p_x2t.tile([P, S], BF16, tag="x2t")
            nc.sync.dma_start(t2[:], x2t[dt * P:(dt + 1) * P, :])
            sb_x2t.append(t2)
        sb_x2n = []
        for jt in range(nT):
            t = p_x2n.tile([P, D], BF16, tag="x2n")
            nc.sync.dma_start(t[:], x2n[jt * P:(jt + 1) * P, :])
            sb_x2n.append(t)
        sb_x1n = []
        for it in range(nT):
            t = p_x1n.tile([P, D], BF16, tag="x1n")
            nc.sync.dma_start(t[:], x1n[it * P:(it + 1) * P, :])
            sb_x1n.append(t)

        # ---------------- phase 1: out1 ----------------
        for ib in range(nIB):
            isl = slice(ib * NB, (ib + 1) * NB)
            e1_tiles = []
            for jt in range(nT):
                ps = ps_sc.tile([P, NB], F32, tag="sc")
                for dt in range(nDT):
                    nc.tensor.matmul(
                        ps[:],
                        sb_x2t[dt][:, jt * P:(jt + 1) * P],
                        sb_x1t[dt][:, isl],
                        start=(dt == 0),
                        stop=(dt == nDT - 1),
                    )
                et = p_e1.tile([P, NB], BF16, tag="e1")
                nc.scalar.activation(
                    et[:], ps[:], AF.Exp, scale=scale,
                    accum_out=d2p[:, jt, ib:ib + 1],
                )
                e1_tiles.append(et)
            for sub in range(nSUB):
                it = ib * nSUB + sub
                ssl = slice(sub * P, (sub + 1) * P)
                ht = p_h.tile([P, D], F32, tag="h")
                nc.sync.dma_start(ht[:], x1h[it * P:(it + 1) * P, :])
                ps_r = ps_dn.tile([P, 1], F32, tag="dn")
                for jt in range(nT):
                    nc.tensor.matmul(
                        ps_r[:], e1_tiles[jt][:, ssl], twos[:],
                        start=(jt == 0), stop=(jt == nT - 1),
                    )
                r1 = p_small.tile([P, 1], F32, tag="r")
                nc.vector.reciprocal(r1[:], ps_r[:])  # = 0.5 / rowsum
                for db in range(nDB):
                    dsl = slice(db * NB, (db + 1) * NB)
                    ps_o = ps_pv.tile([P, NB], F32, tag="pv")
                    for jt in range(nT):
                        nc.tensor.matmul(
                            ps_o[:], e1_tiles[jt][:, ssl], sb_x2n[jt][:, dsl],
                            start=(jt == 0), stop=(jt == nT - 1),
                        )
                    ob = p_out.tile([P, NB], F32, tag="ob")
                    nc.vector.scalar_tensor_tensor(
                        ob[:], ps_o[:], r1[:], ht[:, dsl],
                        op0=ALU.mult, op1=ALU.add,
                    )
                    nc.sync.dma_start(o1[it * P:(it + 1) * P, dsl], ob[:])

        # ---------------- phase 2: out2 (mirror) ----------------
        for jb in range(nIB):
            jsl = slice(jb * NB, (jb + 1) * NB)
            e2_tiles = []
            for it in range(nT):
                ps = ps_sc.tile([P, NB], F32, tag="sc")
                for dt in range(nDT):
                    nc.tensor.matmul(
                        ps[:],
                        sb_x1t[dt][:, it * P:(it + 1) * P],
                        sb_x2t[dt][:, jsl],
                        start=(dt == 0),
                        stop=(dt == nDT - 1),
                    )
                e2 = p_e2.tile([P, NB], BF16, tag="e2")
                nc.scalar.activation(e2[:], ps[:], AF.Exp, scale=scale)
                e2_tiles.append(e2)
            for sub in range(nSUB):
                jt = jb * nSUB + sub
                ssl = slice(sub * P, (sub + 1) * P)
                ht = p_h.tile([P, D], F32, tag="h")
                nc.sync.dma_start(ht[:], x2h[jt * P:(jt + 1) * P, :])
                # denom2 = sum over ib of phase-1 accum partials; r2 = 0.5/denom2
                dn2 = p_small.tile([P, 1], F32, tag="dn2")
                nc.vector.tensor_reduce(
                    dn2[:], d2p[:, jt, :], axis=mybir.AxisListType.X, op=ALU.add,
                )
                dn2x2 = p_small.tile([P, 1], F32, tag="dn2x2")
                nc.vector.tensor_scalar_mul(dn2x2[:], dn2[:], 2.0)
                r2 = p_small.tile([P, 1], F32, tag="r2")
                nc.vector.reciprocal(r2[:], dn2x2[:])
                for db in range(nDB):
                    dsl = slice(db * NB, (db + 1) * NB)
                    ps_o = ps_pv.tile([P, NB], F32, tag="pv")
                    for it in range(nT):
                        nc.tensor.matmul(
                            ps_o[:], e2_tiles[it][:, ssl], sb_x1n[it][:, dsl],
                            start=(it == 0), stop=(it == nT - 1),
                        )
                    ob = p_out.tile([P, NB], F32, tag="ob")
                    nc.vector.scalar_tensor_tensor(
                        ob[:], ps_o[:], r2[:], ht[:, dsl],
                        op0=ALU.mult, op1=ALU.add,
                    )
                    nc.sync.dma_start(o2[jt * P:(jt + 1) * P, dsl], ob[:])


def build_body_v2(nc, tc, S, D, NB=512, io=None):
    """v2: compute scores/exp once (as E^T), produce phase-2's E-natural tiles
    via DMA xbar transpose instead of recomputing the score matmuls.

    PE work drops from 4 to 3 big matmul sets (scores once + two PV passes);
    the transpose rides on the DMA engines (~16MB through the xbar).
    """
    assert S % NB == 0 and D % NB == 0 and S % P == 0 and D % P == 0
    nT = S // P
    nDT = D // P
    nIB = S // NB
    nDB = D // NB
    nSUB = NB // P
    scale = 1.0 / float(np.sqrt(D))

    if io is None:
        io = declare_io(nc, S, D)
    x1t, x2t, x1n, x2n, x1h, x2h, o1, o2 = (
        io["x1t"], io["x2t"], io["x1n"], io["x2n"],
        io["x1h"], io["x2h"], io["o1"], io["o2"],
    )

    with (
        tc.tile_pool(name="p_et", bufs=nT) as p_et,
        tc.tile_pool(name="p_h", bufs=3) as p_h,
        tc.tile_pool(name="p_out", bufs=4) as p_out,
        tc.tile_pool(name="p_small", bufs=8) as p_small,
        tc.tile_pool(name="p_const", bufs=2) as p_const,
        tc.tile_pool(name="ps_sc", bufs=2, space=bass.MemorySpace.PSUM) as ps_sc,
        tc.tile_pool(name="ps_pv", bufs=4, space=bass.MemorySpace.PSUM) as ps_pv,
        tc.tile_pool(name="ps_dn", bufs=2, space=bass.MemorySpace.PSUM) as ps_dn,
    ):
        twos = p_const.tile([P, 1], BF16, tag="twos")
        nc.vector.memset(twos[:], 2.0)
        d2p = p_const.tile([P, nT, nIB], F32, tag="d2p")

        # E^T store: per j-tile, all i columns (written slice-wise by the exps)
        sb_et = [p_et.tile([P, S], BF16, tag="et", name=f"et{j}") for j in range(nT)]

        # ---------------- phase 1 (+ E^T materialization) ----------------
        with (
            tc.tile_pool(name="p_x1t", bufs=nDT) as p_x1t,
            tc.tile_pool(name="p_x2t", bufs=nDT) as p_x2t,
            tc.tile_pool(name="p_x2n", bufs=nT) as p_x2n,
        ):
            sb_x1t, sb_x2t = [], []
            for dt in range(nDT):
                t1 = p_x1t.tile([P, S], BF16, tag="x1t")
                nc.sync.dma_start(t1[:], x1t[dt * P:(dt + 1) * P, :])
                sb_x1t.append(t1)
                t2 = p_x2t.tile([P, S], BF16, tag="x2t")
                nc.sync.dma_start(t2[:], x2t[dt * P:(dt + 1) * P, :])
                sb_x2t.append(t2)
            sb_x2n = []
            for jt in range(nT):
                t = p_x2n.tile([P, D], BF16, tag="x2n")
                nc.sync.dma_start(t[:], x2n[jt * P:(jt + 1) * P, :])
                sb_x2n.append(t)

            for ib in range(nIB):
                isl = slice(ib * NB, (ib + 1) * NB)
                for jt in range(nT):
                    ps = ps_sc.tile([P, NB], F32, tag="sc")
                    for dt in range(nDT):
                        nc.tensor.matmul(
                            ps[:],
                            sb_x2t[dt][:, jt * P:(jt + 1) * P],
                            sb_x1t[dt][:, isl],
                            start=(dt == 0),
                            stop=(dt == nDT - 1),
                        )
                    nc.scalar.activation(
                        sb_et[jt][:, isl], ps[:], AF.Exp, scale=scale,
                        accum_out=d2p[:, jt, ib:ib + 1],
                    )
                for sub in range(nSUB):
                    it = ib * nSUB + sub
                    csl = slice(it * P, (it + 1) * P)
                    ht = p_h.tile([P, D], F32, tag="h")
                    nc.sync.dma_start(ht[:], x1h[it * P:(it + 1) * P, :])
                    # denominator matmuls ride between the PV matmuls so their
                    # ldweights (same stationary operand) stay hidden under the
                    # 512-wide PV streams
                    ps_r = ps_dn.tile([P, 1], F32, tag="dn")
                    ps_os = [
                        ps_pv.tile([P, NB], F32, tag="pv", name=f"pv{db}")
                        for db in range(nDB)
                    ]
                    for jt in range(nT):
                        for db in range(nDB):
                            nc.tensor.matmul(
                                ps_os[db][:], sb_et[jt][:, csl],
                                sb_x2n[jt][:, db * NB:(db + 1) * NB],
                                start=(jt == 0), stop=(jt == nT - 1),
                            )
                        nc.tensor.matmul(
                            ps_r[:], sb_et[jt][:, csl], twos[:],
                            start=(jt == 0), stop=(jt == nT - 1),
                        )
                    r1 = p_small.tile([P, 1], F32, tag="r")
                    nc.vector.reciprocal(r1[:], ps_r[:])  # = 0.5 / rowsum
                    for db in range(nDB):
                        dsl = slice(db * NB, (db + 1) * NB)
                        ob = p_out.tile([P, NB], F32, tag="ob")
                        nc.vector.scalar_tensor_tensor(
                            ob[:], ps_os[db][:], r1[:], ht[:, dsl],
                            op0=ALU.mult, op1=ALU.add,
                        )
                        nc.sync.dma_start(o1[it * P:(it + 1) * P, dsl], ob[:])

        # ---------------- phase 2: transpose E^T, PV with X1 ----------------
        with (
            tc.tile_pool(name="p_x1n", bufs=nT) as p_x1n,
            tc.tile_pool(name="p_en", bufs=3) as p_en,
        ):
            sb_x1n = []
            for it in range(nT):
                t = p_x1n.tile([P, D], BF16, tag="x1n")
                nc.sync.dma_start(t[:], x1n[it * P:(it + 1) * P, :])
                sb_x1n.append(t)

            for jt in range(nT):
                # e_nat[p_i, kt, f_j] = E[kt*128+p_i, jt*128+f_j]
                en = p_en.tile([P, nT, P], BF16, tag="en")
                nc.sync.dma_start_transpose(en[:], sb_et[jt][:])

                dn2 = p_small.tile([P, 1], F32, tag="dn2")
                nc.vector.tensor_reduce(
                    dn2[:], d2p[:, jt, :], axis=mybir.AxisListType.X, op=ALU.add,
                )
                dn2x2 = p_small.tile([P, 1], F32, tag="dn2x2")
                nc.vector.tensor_scalar_mul(dn2x2[:], dn2[:], 2.0)
                r2 = p_small.tile([P, 1], F32, tag="r2")
                nc.vector.reciprocal(r2[:], dn2x2[:])  # = 0.5 / colsum

                ht = p_h.tile([P, D], F32, tag="h")
                nc.sync.dma_start(ht[:], x2h[jt * P:(jt + 1) * P, :])
                for db in range(nDB):
                    dsl = slice(db * NB, (db + 1) * NB)
                    ps_o = ps_pv.tile([P, NB], F32, tag="pv")
                    for it in range(nT):
                        nc.tensor.matmul(
                            ps_o[:], en[:, it, :], sb_x1n[it][:, dsl],
                            start=(it == 0), stop=(it == nT - 1),
                        )
                    ob = p_out.tile([P, NB], F32, tag="ob")
                    nc.vector.scalar_tensor_tensor(
                        ob[:], ps_o[:], r2[:], ht[:, dsl],
                        op0=ALU.mult, op1=ALU.add,
                    )
                    nc.sync.dma_start(o2[jt * P:(jt + 1) * P, dsl], ob[:])


BODY_VERSION = 2


def build_nc(S=2048, D=1024, NB=512, n_cores=8, repeats=1, version=None):
    nc = bacc.Bacc(
        "TRN2",
        target_bir_lowering=False,
        debug=False,
        enable_asserts=False,
        num_devices=n_cores,
    )
    if version is None:
        version = BODY_VERSION
    body = {1: build_body, 2: build_body_v2}[version]
    with tile.TileContext(nc) as tc:
        io = declare_io(nc, S, D)
        # benchmark mode (repeats > 1): emit the body R times back-to-back so
        # per-execution device time can be measured as a slope over R
        for _ in range(repeats):
            body(nc, tc, S, D, NB, io=io)
    nc.compile()
    return nc


def make_in_map(x1, x2):
    """Host-side prep of one batch element's per-core inputs."""
    x1 = np.ascontiguousarray(x1, dtype=np.float32)
    x2 = np.ascontiguousarray(x2, dtype=np.float32)
    return {
        "x1t": np.ascontiguousarray(x1.T).astype(ml_dtypes.bfloat16),
        "x2t": np.ascontiguousarray(x2.T).astype(ml_dtypes.bfloat16),
        "x1n": x1.astype(ml_dtypes.bfloat16),
        "x2n": x2.astype(ml_dtypes.bfloat16),
        "x1h": (0.5 * x1).astype(np.float32),
        "x2h": (0.5 * x2).astype(np.float32),
    }


_NC_CACHE = {}


def _get_nc(S, D, n_cores):
    key = (S, D, n_cores)
    if key not in _NC_CACHE:
        _NC_CACHE[key] = build_nc(S=S, D=D, n_cores=n_cores)
    return _NC_CACHE[key]


def kernel(layer_key=None, input1=None, input2=None, _trace=False, **_ignored):
    X1 = np.asarray(input1, dtype=np.float32)
    X2 = np.asarray(input2, dtype=np.float32)
    B, S, D = X1.shape
    n_cores = 8
    assert B == n_cores, f"expected batch {n_cores}, got {B}"

    nc = _get_nc(S, D, n_cores)
    in_maps = [make_in_map(X1[b], X2[b]) for b in range(B)]
    res = run_bass_kernel_spmd(
        nc, in_maps, core_ids=list(range(n_cores)),
        trace=_trace, trace_cores=[0] if _trace else None,
    )
    out1 = np.stack([res.results[b]["o1"] for b in range(B)])
    out2 = np.stack([res.results[b]["o2"] for b in range(B)])
    if _trace:
        kernel.last_results = res
    return (out1, out2)
